# revision 27
# baseline (speedup 1.0000x reference)
"""Bass/Trainium2 kernel for additive (Bahdanau-style) attention.

  query  = decoder_hidden @ Wq                      [B, H]
  keys   = encoder_outputs @ Wk                     [B, S, H]
  energy = tanh(query[:, None, :] + keys) @ We      [B, S, 1]
  attn   = softmax(energy, axis=1)                  [B, S, 1]
  context= attn^T @ encoder_outputs                 [B, H]

Shapes: B=32, S=1024, H=1024 (fp32). Data-parallel over batch across the 8
NeuronCores (4 batches per core); weights replicated.

Per-core dataflow:
  - the keys matmul dominates (2*B*S*H*H/8 = 8.6 GFLOP/core). Moving-operand
    SBUF bandwidth makes 4-byte matmuls stream at ~2 cycles/row, so the PE
    computes in fp16 (10-bit mantissa: ~4e-4 end-to-end error; PSUM
    accumulation stays fp32).
  - E_b is DMA'd in natural layout, cast fp32->fp16 by ACT/DVE copies, then
    PE-transposed per 128x128 block into E^T (the keys contraction is over h,
    which must sit on partitions).
  - keys^T[d,s] accumulates in PSUM over 8 k-tiles; ACT applies
    tanh(keys^T + q^T) with the query as per-partition bias (query itself is
    computed once for all 4 batches in fp32r).
  - batch 0 runs before Wq can arrive, so its keys^T tiles are spilled to SBUF
    and the tanh+energy pass runs after the queries — this keeps the PE busy
    during the unavoidable ~12 MB cold-start DMA (E0+Wk+Wq).
  - energy row [1, S] via M=1 matmuls with We as stationary; softmax without
    max-subtraction (energies are O(1)); normalization folded as a 1/Z scale
    into the context/attn outputs.
  - context row via M=1 matmuls of attn^T against natural-layout fp16 E_b.
"""

import numpy as np

import concourse.bass as bass
import concourse.tile as tile
from concourse import bacc, mybir
from concourse.bass_utils import run_bass_kernel_spmd
from concourse.masks import make_identity
from concourse.tile_rust import add_dep_helper

B, S, H = 32, 1024, 1024
NCORES = 8
BPC = B // NCORES  # batches per core
KT = H // 128      # contraction tiles
ST = S // 128      # s tiles

F32 = mybir.dt.float32
F32R = mybir.dt.float32r
F16 = mybir.dt.float16
AF = mybir.ActivationFunctionType

_BUILT = {}
TRACE = False


def _build():
    nc = bacc.Bacc()

    dh = nc.dram_tensor("decoder_hidden", [BPC, H], F32, kind="ExternalInput")
    enc = nc.dram_tensor("encoder_outputs", [BPC, S, H], F32, kind="ExternalInput")
    wq_d = nc.dram_tensor("Wq", [H, H], F32, kind="ExternalInput")
    wk_d = nc.dram_tensor("Wk", [H, H], F32, kind="ExternalInput")
    we_d = nc.dram_tensor("We", [H, 1], F32, kind="ExternalInput")
    ctx_out = nc.dram_tensor("context", [BPC, H], F32, kind="ExternalOutput")
    attn_out = nc.dram_tensor("attn", [BPC, S, 1], F32, kind="ExternalOutput")

    with tile.TileContext(nc) as tc:
        with (
            tc.tile_pool(name="const", bufs=1) as const,
            tc.tile_pool(name="wk_pool", bufs=1) as wk_pool,
            tc.tile_pool(name="wq_pool", bufs=1) as wq_pool,
            tc.tile_pool(name="et_pool", bufs=1) as et_pool,
            tc.tile_pool(name="enat", bufs=2) as enat_pool,
            tc.tile_pool(name="enat16", bufs=2) as enat16_pool,
            tc.tile_pool(name="tpool", bufs=4) as tpool,
            tc.tile_pool(name="rows", bufs=4) as rows,
            tc.tile_pool(name="ps_tr", bufs=2, space="PSUM") as ps_tr,
            tc.tile_pool(name="ps_keys", bufs=4, space="PSUM") as ps_keys,
            tc.tile_pool(name="ps_row", bufs=1, space="PSUM") as ps_row,
        ):
            # --- small input DMAs first so they aren't stuck behind the MBs
            we_row = rows.tile([1, H], F32, tag="rows")
            nc.sync.dma_start(out=we_row[:], in_=we_d[:, :].rearrange("s o -> o s"))
            dh_sb = rows.tile([BPC, H], F32R, tag="rows")
            nc.sync.dma_start(out=dh_sb[:], in_=dh[:, :].bitcast(F32R))

            # DMA order on the sync queue: E0 (its on-chip cast+transpose is
            # the first PE work), Wk (keys gate on it), Wq, then E1..E3. wk32
            # is staging only, so it shares the e_nat pool slots (released
            # after the fp16 cast, before e_nat1 needs the slot).
            def load_e(b, nsplit=2):
                t = enat_pool.tile([128, ST, H], F32, tag="enat", name=f"e_nat{b}")
                src = enc[b:b + 1, :, :].rearrange("o (st p) h -> p (o st) h", p=128)
                step = ST // nsplit
                for i in range(nsplit):
                    nc.sync.dma_start(out=t[:, i * step:(i + 1) * step, :],
                                      in_=src[:, i * step:(i + 1) * step, :])
                return t

            e_nats = [None] * BPC
            e_nats[0] = load_e(0, nsplit=4)
            wk32 = enat_pool.tile([128, KT, H], F32, tag="enat", name="wk32")
            wk_src = wk_d[:, :].rearrange("(k p) d -> p k d", p=128)
            for i in range(4):
                nc.sync.dma_start(out=wk32[:, 2 * i:2 * i + 2, :],
                                  in_=wk_src[:, 2 * i:2 * i + 2, :])
            wq = wq_pool.tile([128, KT, H], F32R)
            wq_src = wq_d[:, :].rearrange("(k p) d -> p k d", p=128).bitcast(F32R)
            for i in range(4):
                nc.sync.dma_start(out=wq[:, 2 * i:2 * i + 2, :],
                                  in_=wq_src[:, 2 * i:2 * i + 2, :])
            for b in range(1, BPC):
                e_nats[b] = load_e(b)

            ident_f = const.tile([128, 128], F32)
            make_identity(nc, ident_f[:])
            ident16 = const.tile([128, 128], F16)
            nc.vector.tensor_copy(ident16[:], ident_f[:])
            sel4 = const.tile([128, 1], F16)
            nc.vector.tensor_add(sel4[:], ident16[:, 0:1], ident16[:, 32:33])
            nc.vector.tensor_add(sel4[:], sel4[:], ident16[:, 64:65])
            nc.vector.tensor_add(sel4[:], sel4[:], ident16[:, 96:97])

            # Wk cast to fp16 (one-time; releases wk32's pool slot)
            wk = wk_pool.tile([128, KT, H], F16, tag="wk16")
            for k in range(KT):
                nc.vector.tensor_copy(wk[:, k, :], wk32[:, k, :])

            # We row -> [128, KT] fp16 columns (K=1 fp32 matmul vs identity)
            we_sb = const.tile([128, KT], F16)
            for k in range(KT):
                pst = ps_tr.tile([128, 1], F32, tag="pst")
                nc.tensor.matmul(
                    pst[:], we_row[0:1, k * 128:(k + 1) * 128], ident_f[0:1, 0:1],
                    start=True, stop=True,
                )
                nc.vector.tensor_copy(we_sb[:, k:k + 1], pst[:])

            # dh^T tiles [128, BPC] per k-tile (fp32r, for the fp32r query mm)
            dhT = const.tile([128, KT, BPC], F32R)
            for k in range(KT):
                pst = ps_tr.tile([128, BPC], F32, tag="pst")
                nc.tensor.matmul(
                    pst[:], dh_sb[0:BPC, k * 128:(k + 1) * 128].bitcast(F32),
                    ident_f[0:BPC, 0:BPC],
                    start=True, stop=True,
                )
                nc.vector.tensor_copy(dhT[:, k, :], pst[:])

            def emit_e16_cast(e_nat, e16, sts):
                # fp32 -> fp16 cast of e_nat s-tiles; alternate ACT/DVE
                for i, st in enumerate(sts):
                    if i % 2 == 0:
                        nc.scalar.copy(e16[:, st, :], e_nat[:, st, :])
                    else:
                        nc.vector.tensor_copy(e16[:, st, :], e_nat[:, st, :])

            def emit_eT_transposes(e16, eT, g):
                # e16 [128(s), ST, H] -> eT [128(h), KT, S] for s-half g
                for k in range(KT):
                    pst = ps_tr.tile([128, 512], F16, tag="pst")
                    for j in range(4):
                        st = g * 4 + j
                        nc.tensor.transpose(
                            pst[:, j * 128:(j + 1) * 128],
                            e16[:, st, k * 128:(k + 1) * 128],
                            ident16[:],
                        )
                    nc.vector.tensor_copy(eT[:, k, g * 512:(g + 1) * 512], pst[:])

            def emit_prep(b, eT):
                e16 = enat16_pool.tile([128, ST, H], F16, tag="e16", name=f"e16_{b}")
                for g in range(2):
                    emit_e16_cast(e_nats[b], e16, range(g * 4, (g + 1) * 4))
                    emit_eT_transposes(e16, eT, g)
                return e16

            def emit_energy(p):
                out_, lhsT_, rhs_, start_, stop_ = p
                return nc.tensor.matmul(
                    out_, lhsT_, rhs_, start=start_, stop=stop_,
                    skip_group_check=True,
                )

            STEPS = [(h, m) for h in range(2) for m in range(KT)]

            def emit_keys(eT, h, m):
                kp = ps_keys.tile([128, 512], F32, tag="keys")
                last = None
                for k in range(KT):
                    last = nc.tensor.matmul(
                        kp[:],
                        wk[:, k, m * 128:(m + 1) * 128],
                        eT[:, k, h * 512:(h + 1) * 512],
                        start=(k == 0),
                        stop=(k == KT - 1),
                    )
                return kp, last

            # ---- batch 0 prep + keys (keys^T spilled to SBUF: tanh/energy
            # wait for Wq, which is still ~25us out when these run)
            e16s = [None] * BPC
            eTs = [None] * BPC
            eTs[0] = et_pool.tile([128, KT, S], F16, tag="eT", name="eT0")
            e16s[0] = emit_prep(0, eTs[0])

            keys_sb0 = enat16_pool.tile([128, 2 * KT, 512], F16, tag="e16",
                                        name="keys_sb0")
            last_b0_keys = None
            for idx, (h, m) in enumerate(STEPS):
                kp, last_b0_keys = emit_keys(eTs[0], h, m)
                if idx % 2 == 0:
                    nc.scalar.copy(keys_sb0[:, h * KT + m, :], kp[:])
                else:
                    nc.vector.tensor_copy(keys_sb0[:, h * KT + m, :], kp[:])

            # ---- queries for all batches: Q[BPC, H] = dh @ Wq (fp32r),
            # ordered after batch 0's keys so the PE stream never stalls on
            # the Wq DMA
            q_ps = ps_row.tile([BPC, H], F32, tag="ps_row")
            first_q = None
            for k in range(KT):
                for nh in range(2):
                    mm = nc.tensor.matmul(
                        q_ps[:, nh * 512:(nh + 1) * 512],
                        dhT[:, k, :],
                        wq[:, k, nh * 512:(nh + 1) * 512],
                        start=(k == 0),
                        stop=(k == KT - 1),
                        skip_group_check=True,
                    )
                    if first_q is None:
                        first_q = mm
            add_dep_helper(
                first_q.ins, last_b0_keys.ins, sync=False,
                reason="PE stream order: b0 keys before queries (Wq DMA is late)",
            )
            q_sb = rows.tile([BPC, H], F32, tag="rows")
            nc.vector.tensor_copy(q_sb[:], q_ps[:])
            # q^T: per m-tile, per batch: [128, 1] fp32 bias columns
            qT = const.tile([128, KT, BPC], F32)
            for m in range(KT):
                pst = ps_tr.tile([128, BPC], F32, tag="pst")
                nc.tensor.matmul(
                    pst[:], q_sb[0:BPC, m * 128:(m + 1) * 128], ident_f[0:BPC, 0:BPC],
                    start=True, stop=True,
                )
                nc.vector.tensor_copy(qT[:, m, :], pst[:])

            for b in range(BPC):
                e16 = e16s[b]
                eT = eTs[b]

                # keys^T -> tanh -> energy, with lag-1 energy matmuls so the
                # PE never waits on the ACT tanh of the current m-tile
                energy_ps = ps_row.tile([1, S], F32, tag="ps_row", name=f"energy_ps{b}")
                pending = None
                for h, m in STEPS:
                    sl = slice(h * 512, (h + 1) * 512)
                    if b == 0:
                        tanh_in = keys_sb0[:, h * KT + m, :]
                    else:
                        kp, _ = emit_keys(eT, h, m)
                        tanh_in = kp[:]
                    t_sb = tpool.tile([128, 512], F16, tag="tanh")
                    nc.scalar.activation(
                        t_sb[:], tanh_in, AF.Tanh, bias=qT[:, m, :][:, b:b + 1]
                    )
                    if pending is not None:
                        emit_energy(pending)
                    pending = (
                        energy_ps[0:1, sl], we_sb[:, m:m + 1], t_sb[:],
                        m == 0, m == KT - 1,
                    )
                emit_energy(pending)

                # softmax: exp (no max subtraction; energies are O(1)).
                # fp16 output halves: the attn^T transposes then run as
                # single-pass K=1 fp16 matmuls (fp32 K=1 decomposes into 2)
                attn_u = rows.tile([1, S], F16, tag="attn_u")
                nc.scalar.activation(attn_u[0:1, 0:512], energy_ps[0:1, 0:512], AF.Exp)
                nc.scalar.activation(attn_u[0:1, 512:1024], energy_ps[0:1, 512:1024], AF.Exp)
                zt = const.tile([1, 1], F32, tag=f"z{b}")
                nc.vector.reduce_sum(zt[:], attn_u[:], axis=mybir.AxisListType.X)
                rz = const.tile([1, 1], F32, tag=f"rz{b}")
                nc.vector.reciprocal(rz[:], zt[:])
                attn_o = rows.tile([1, S], F32, tag="rows")
                nc.vector.tensor_scalar_mul(attn_o[:], attn_u[:], rz[0:1, 0:1])
                nc.scalar.dma_start(
                    out=attn_out[b:b + 1, :, :].rearrange("o s x -> (o x) s"),
                    in_=attn_o[:],
                )

                # next batch's cast+transposes keep the PE busy while softmax
                # finishes on ACT/DVE
                if b + 1 < BPC:
                    eTs[b + 1] = et_pool.tile([128, KT, S], F16, tag="eT",
                                              name=f"eT{b+1}")
                    e16s[b + 1] = emit_prep(b + 1, eTs[b + 1])

                # attn^T columns for the context matmul (K=1 fp16 mm)
                attnT = const.tile([128, ST], F16, tag=f"attnT{b}")
                for st in range(ST):
                    pst = ps_tr.tile([128, 1], F32, tag="pst")
                    nc.tensor.matmul(
                        pst[:], attn_u[0:1, st * 128:(st + 1) * 128], ident16[0:1, 0:1],
                        start=True, stop=True,
                    )
                    nc.scalar.copy(attnT[:, st:st + 1], pst[:])

                ctx_ps = ps_row.tile([1, H], F32, tag="ps_row")
                for nh in range(2):
                    nsl = slice(nh * 512, (nh + 1) * 512)
                    ctx_part = ps_keys.tile([128, 512], F32, tag="keys",
                                            name=f"ctxp{b}_{nh}")
                    for r in range(2):
                        for j in range(4):
                            st = r * 4 + j
                            nc.tensor.matmul(
                                ctx_part[32 * j:32 * j + 1, :],
                                attnT[:, st:st + 1],
                                e16[:, st, nsl],
                                start=(r == 0), stop=(r == 1),
                                tile_position=(0, 32 * j),
                                skip_group_check=True,
                            )
                    ctx_sb = tpool.tile([128, 512], F16, tag="tanh",
                                        name=f"ctxs{b}_{nh}")
                    nc.vector.tensor_copy(ctx_sb[:], ctx_part[:])
                    nc.tensor.matmul(
                        ctx_ps[0:1, nsl], sel4[:], ctx_sb[:],
                        start=True, stop=True, skip_group_check=True,
                    )
                ctx_o = rows.tile([1, H], F32, tag="rows")
                nc.vector.tensor_scalar_mul(ctx_o[:], ctx_ps[:], rz[0:1, 0:1])
                nc.scalar.dma_start(out=ctx_out[b:b + 1, :], in_=ctx_o[:])

    nc.finalize()
    return nc


def _get_nc():
    if "nc" not in _BUILT:
        _BUILT["nc"] = _build()
    return _BUILT["nc"]


def kernel(decoder_hidden, encoder_outputs, Wq, Wk, We):
    decoder_hidden = np.ascontiguousarray(np.asarray(decoder_hidden, dtype=np.float32))
    encoder_outputs = np.ascontiguousarray(np.asarray(encoder_outputs, dtype=np.float32))
    Wq = np.ascontiguousarray(np.asarray(Wq, dtype=np.float32))
    Wk = np.ascontiguousarray(np.asarray(Wk, dtype=np.float32))
    We = np.ascontiguousarray(np.asarray(We, dtype=np.float32))

    nc = _get_nc()
    in_maps = []
    for c in range(NCORES):
        lo, hi = c * BPC, (c + 1) * BPC
        in_maps.append({
            "decoder_hidden": decoder_hidden[lo:hi],
            "encoder_outputs": encoder_outputs[lo:hi],
            "Wq": Wq,
            "Wk": Wk,
            "We": We,
        })
    res = run_bass_kernel_spmd(nc, in_maps, core_ids=list(range(NCORES)), trace=TRACE)
    if TRACE:
        _BUILT["last_result"] = res
    context = np.concatenate([res.results[c]["context"] for c in range(NCORES)], axis=0)
    attn = np.concatenate([res.results[c]["attn"] for c in range(NCORES)], axis=0)
    return (context, attn)


# revision 28
# speedup vs baseline: 1.0579x; 1.0579x over previous
"""Bass/Trainium2 kernel for additive (Bahdanau-style) attention.

  query  = decoder_hidden @ Wq                      [B, H]
  keys   = encoder_outputs @ Wk                     [B, S, H]
  energy = tanh(query[:, None, :] + keys) @ We      [B, S, 1]
  attn   = softmax(energy, axis=1)                  [B, S, 1]
  context= attn^T @ encoder_outputs                 [B, H]

Shapes: B=32, S=1024, H=1024 (fp32). Data-parallel over batch across the 8
NeuronCores (4 batches per core); weights replicated.

Per-core dataflow:
  - the keys matmul dominates (2*B*S*H*H/8 = 8.6 GFLOP/core). Moving-operand
    SBUF bandwidth makes 4-byte matmuls stream at ~2 cycles/row, so the PE
    computes in fp16 (10-bit mantissa: ~4e-4 end-to-end error; PSUM
    accumulation stays fp32).
  - E_b is DMA'd in natural layout, cast fp32->fp16 by ACT/DVE copies, then
    PE-transposed per 128x128 block into E^T (the keys contraction is over h,
    which must sit on partitions).
  - keys^T[d,s] accumulates in PSUM over 8 k-tiles; ACT applies
    tanh(keys^T + q^T) with the query as per-partition bias (query itself is
    computed once for all 4 batches in fp32r).
  - batch 0 runs before Wq can arrive, so its keys^T tiles are spilled to SBUF
    and the tanh+energy pass runs after the queries — this keeps the PE busy
    during the unavoidable ~12 MB cold-start DMA (E0+Wk+Wq).
  - energy row [1, S] via M=1 matmuls with We as stationary; softmax without
    max-subtraction (energies are O(1)); normalization folded as a 1/Z scale
    into the context/attn outputs.
  - context row via M=1 matmuls of attn^T against natural-layout fp16 E_b.
"""

import numpy as np

import concourse.bass as bass
import concourse.tile as tile
from concourse import bacc, mybir
from concourse.bass_utils import run_bass_kernel_spmd
from concourse.masks import make_identity
from concourse.tile_rust import add_dep_helper

B, S, H = 32, 1024, 1024
NCORES = 8
BPC = B // NCORES  # batches per core
KT = H // 128      # contraction tiles
ST = S // 128      # s tiles

F32 = mybir.dt.float32
F32R = mybir.dt.float32r
F16 = mybir.dt.float16
AF = mybir.ActivationFunctionType

_BUILT = {}
TRACE = False


def _build():
    nc = bacc.Bacc()

    dh = nc.dram_tensor("decoder_hidden", [BPC, H], F32, kind="ExternalInput")
    enc = nc.dram_tensor("encoder_outputs", [BPC, S, H], F32, kind="ExternalInput")
    wq_d = nc.dram_tensor("Wq", [H, H], F32, kind="ExternalInput")
    wk_d = nc.dram_tensor("Wk", [H, H], F32, kind="ExternalInput")
    we_d = nc.dram_tensor("We", [H, 1], F32, kind="ExternalInput")
    ctx_out = nc.dram_tensor("context", [BPC, H], F32, kind="ExternalOutput")
    attn_out = nc.dram_tensor("attn", [BPC, S, 1], F32, kind="ExternalOutput")

    with tile.TileContext(nc) as tc:
        with (
            tc.tile_pool(name="const", bufs=1) as const,
            tc.tile_pool(name="wk_pool", bufs=1) as wk_pool,
            tc.tile_pool(name="wq_pool", bufs=1) as wq_pool,
            tc.tile_pool(name="et_pool", bufs=1) as et_pool,
            tc.tile_pool(name="enat", bufs=2) as enat_pool,
            tc.tile_pool(name="enat16", bufs=2) as enat16_pool,
            tc.tile_pool(name="tpool", bufs=4) as tpool,
            tc.tile_pool(name="rows", bufs=4) as rows,
            tc.tile_pool(name="ps_tr", bufs=2, space="PSUM") as ps_tr,
            tc.tile_pool(name="ps_keys", bufs=4, space="PSUM") as ps_keys,
            tc.tile_pool(name="ps_row", bufs=1, space="PSUM") as ps_row,
        ):
            # --- small input DMAs first so they aren't stuck behind the MBs
            we_row = rows.tile([1, H], F32, tag="rows")
            nc.sync.dma_start(out=we_row[:], in_=we_d[:, :].rearrange("s o -> o s"))
            dh_sb = rows.tile([BPC, H], F32R, tag="rows")
            nc.sync.dma_start(out=dh_sb[:], in_=dh[:, :].bitcast(F32R))

            # DMA order on the sync queue: E0 (its on-chip cast+transpose is
            # the first PE work), Wk (keys gate on it), Wq, then E1..E3. wk32
            # is staging only, so it shares the e_nat pool slots (released
            # after the fp16 cast, before e_nat1 needs the slot).
            def load_e(b, nsplit=2):
                t = enat_pool.tile([128, ST, H], F32, tag="enat", name=f"e_nat{b}")
                src = enc[b:b + 1, :, :].rearrange("o (st p) h -> p (o st) h", p=128)
                step = ST // nsplit
                for i in range(nsplit):
                    nc.sync.dma_start(out=t[:, i * step:(i + 1) * step, :],
                                      in_=src[:, i * step:(i + 1) * step, :])
                return t

            e_nats = [None] * BPC
            e_nats[0] = load_e(0, nsplit=4)
            wk32 = enat_pool.tile([128, KT, H], F32, tag="enat", name="wk32")
            wk_src = wk_d[:, :].rearrange("(k p) d -> p k d", p=128)
            for i in range(4):
                nc.sync.dma_start(out=wk32[:, 2 * i:2 * i + 2, :],
                                  in_=wk_src[:, 2 * i:2 * i + 2, :])
            wq = wq_pool.tile([128, KT, H], F32R)
            wq_src = wq_d[:, :].rearrange("(k p) d -> p k d", p=128).bitcast(F32R)
            for i in range(4):
                nc.sync.dma_start(out=wq[:, 2 * i:2 * i + 2, :],
                                  in_=wq_src[:, 2 * i:2 * i + 2, :])
            for b in range(1, BPC):
                e_nats[b] = load_e(b)

            ident_f = const.tile([128, 128], F32)
            make_identity(nc, ident_f[:])
            ident16 = const.tile([128, 128], F16)
            nc.vector.tensor_copy(ident16[:], ident_f[:])

            # Wk cast to fp16 (one-time; releases wk32's pool slot)
            wk = wk_pool.tile([128, KT, H], F16, tag="wk16")
            for k in range(KT):
                nc.vector.tensor_copy(wk[:, k, :], wk32[:, k, :])

            # We row -> [128, KT] fp16 columns (K=1 fp32 matmul vs identity)
            we_sb = const.tile([128, KT], F16)
            for k in range(KT):
                pst = ps_tr.tile([128, 1], F32, tag="pst")
                nc.tensor.matmul(
                    pst[:], we_row[0:1, k * 128:(k + 1) * 128], ident_f[0:1, 0:1],
                    start=True, stop=True,
                )
                nc.vector.tensor_copy(we_sb[:, k:k + 1], pst[:])

            # dh^T tiles [128, BPC] per k-tile (fp32r, for the fp32r query mm)
            dhT = const.tile([128, KT, BPC], F32R)
            for k in range(KT):
                pst = ps_tr.tile([128, BPC], F32, tag="pst")
                nc.tensor.matmul(
                    pst[:], dh_sb[0:BPC, k * 128:(k + 1) * 128].bitcast(F32),
                    ident_f[0:BPC, 0:BPC],
                    start=True, stop=True,
                )
                nc.vector.tensor_copy(dhT[:, k, :], pst[:])

            def emit_e16_cast(e_nat, e16, sts):
                # fp32 -> fp16 cast of e_nat s-tiles; alternate ACT/DVE
                for i, st in enumerate(sts):
                    if i % 2 == 0:
                        nc.scalar.copy(e16[:, st, :], e_nat[:, st, :])
                    else:
                        nc.vector.tensor_copy(e16[:, st, :], e_nat[:, st, :])

            def emit_eT_transposes(e16, eT, g):
                # e16 [128(s), ST, H] -> eT [128(h), KT, S] for s-half g
                for k in range(KT):
                    pst = ps_tr.tile([128, 512], F16, tag="pst")
                    for j in range(4):
                        st = g * 4 + j
                        nc.tensor.transpose(
                            pst[:, j * 128:(j + 1) * 128],
                            e16[:, st, k * 128:(k + 1) * 128],
                            ident16[:],
                        )
                    nc.vector.tensor_copy(eT[:, k, g * 512:(g + 1) * 512], pst[:])

            def emit_prep(b, eT):
                e16 = enat16_pool.tile([128, ST, H], F16, tag="e16", name=f"e16_{b}")
                for g in range(2):
                    emit_e16_cast(e_nats[b], e16, range(g * 4, (g + 1) * 4))
                    emit_eT_transposes(e16, eT, g)
                return e16

            def emit_energy(p):
                out_, lhsT_, rhs_, start_, stop_ = p
                return nc.tensor.matmul(
                    out_, lhsT_, rhs_, start=start_, stop=stop_,
                    skip_group_check=True,
                )

            STEPS = [(h, m) for h in range(2) for m in range(KT)]

            def emit_keys(eT, h, m):
                kp = ps_keys.tile([128, 512], F32, tag="keys")
                last = None
                for k in range(KT):
                    last = nc.tensor.matmul(
                        kp[:],
                        wk[:, k, m * 128:(m + 1) * 128],
                        eT[:, k, h * 512:(h + 1) * 512],
                        start=(k == 0),
                        stop=(k == KT - 1),
                    )
                return kp, last

            # ---- batch 0 prep + keys (keys^T spilled to SBUF: tanh/energy
            # wait for Wq, which is still ~25us out when these run)
            e16s = [None] * BPC
            eTs = [None] * BPC
            eTs[0] = et_pool.tile([128, KT, S], F16, tag="eT", name="eT0")
            e16s[0] = emit_prep(0, eTs[0])

            keys_sb0 = enat16_pool.tile([128, 2 * KT, 512], F16, tag="e16",
                                        name="keys_sb0")
            last_b0_keys = None
            for idx, (h, m) in enumerate(STEPS):
                kp, last_b0_keys = emit_keys(eTs[0], h, m)
                if idx % 2 == 0:
                    nc.scalar.copy(keys_sb0[:, h * KT + m, :], kp[:])
                else:
                    nc.vector.tensor_copy(keys_sb0[:, h * KT + m, :], kp[:])

            # ---- queries for all batches: Q[BPC, H] = dh @ Wq (fp32r),
            # ordered after batch 0's keys so the PE stream never stalls on
            # the Wq DMA
            q_ps = ps_row.tile([BPC, H], F32, tag="ps_row")
            first_q = None
            for k in range(KT):
                for nh in range(2):
                    mm = nc.tensor.matmul(
                        q_ps[:, nh * 512:(nh + 1) * 512],
                        dhT[:, k, :],
                        wq[:, k, nh * 512:(nh + 1) * 512],
                        start=(k == 0),
                        stop=(k == KT - 1),
                        skip_group_check=True,
                    )
                    if first_q is None:
                        first_q = mm
            add_dep_helper(
                first_q.ins, last_b0_keys.ins, sync=False,
                reason="PE stream order: b0 keys before queries (Wq DMA is late)",
            )
            q_sb = rows.tile([BPC, H], F32, tag="rows")
            nc.vector.tensor_copy(q_sb[:], q_ps[:])
            # q^T: per m-tile, per batch: [128, 1] fp32 bias columns
            qT = const.tile([128, KT, BPC], F32)
            for m in range(KT):
                pst = ps_tr.tile([128, BPC], F32, tag="pst")
                nc.tensor.matmul(
                    pst[:], q_sb[0:BPC, m * 128:(m + 1) * 128], ident_f[0:BPC, 0:BPC],
                    start=True, stop=True,
                )
                nc.vector.tensor_copy(qT[:, m, :], pst[:])

            for b in range(BPC):
                e16 = e16s[b]
                eT = eTs[b]

                # keys^T -> tanh -> energy, with lag-1 energy matmuls so the
                # PE never waits on the ACT tanh of the current m-tile
                energy_ps = ps_row.tile([1, S], F32, tag="ps_row", name=f"energy_ps{b}")
                pending = None
                for h, m in STEPS:
                    sl = slice(h * 512, (h + 1) * 512)
                    if b == 0:
                        tanh_in = keys_sb0[:, h * KT + m, :]
                    else:
                        kp, _ = emit_keys(eT, h, m)
                        tanh_in = kp[:]
                    t_sb = tpool.tile([128, 512], F16, tag="tanh")
                    nc.scalar.activation(
                        t_sb[:], tanh_in, AF.Tanh, bias=qT[:, m, :][:, b:b + 1]
                    )
                    if pending is not None:
                        emit_energy(pending)
                    pending = (
                        energy_ps[0:1, sl], we_sb[:, m:m + 1], t_sb[:],
                        m == 0, m == KT - 1,
                    )
                emit_energy(pending)

                # softmax: exp (no max subtraction; energies are O(1)).
                # fp16 output halves: the attn^T transposes then run as
                # single-pass K=1 fp16 matmuls (fp32 K=1 decomposes into 2)
                attn_u = rows.tile([1, S], F16, tag="attn_u")
                nc.scalar.activation(attn_u[0:1, 0:512], energy_ps[0:1, 0:512], AF.Exp)
                nc.scalar.activation(attn_u[0:1, 512:1024], energy_ps[0:1, 512:1024], AF.Exp)
                zt = const.tile([1, 1], F32, tag=f"z{b}")
                nc.vector.reduce_sum(zt[:], attn_u[:], axis=mybir.AxisListType.X)
                rz = const.tile([1, 1], F32, tag=f"rz{b}")
                nc.vector.reciprocal(rz[:], zt[:])
                attn_o = rows.tile([1, S], F32, tag="rows")
                nc.vector.tensor_scalar_mul(attn_o[:], attn_u[:], rz[0:1, 0:1])
                nc.scalar.dma_start(
                    out=attn_out[b:b + 1, :, :].rearrange("o s x -> (o x) s"),
                    in_=attn_o[:],
                )

                # next batch's cast+transposes keep the PE busy while softmax
                # finishes on ACT/DVE
                if b + 1 < BPC:
                    eTs[b + 1] = et_pool.tile([128, KT, S], F16, tag="eT",
                                              name=f"eT{b+1}")
                    e16s[b + 1] = emit_prep(b + 1, eTs[b + 1])

                # attn^T columns for the context matmul (K=1 fp16 mm)
                attnT = const.tile([128, ST], F16, tag=f"attnT{b}")
                for st in range(ST):
                    pst = ps_tr.tile([128, 1], F32, tag="pst")
                    nc.tensor.matmul(
                        pst[:], attn_u[0:1, st * 128:(st + 1) * 128], ident16[0:1, 0:1],
                        start=True, stop=True,
                    )
                    nc.scalar.copy(attnT[:, st:st + 1], pst[:])

                ctx_ps = ps_row.tile([1, H], F32, tag="ps_row")
                for st in range(ST):
                    for nh in range(2):
                        nc.tensor.matmul(
                            ctx_ps[0:1, nh * 512:(nh + 1) * 512],
                            attnT[:, st:st + 1],
                            e16[:, st, nh * 512:(nh + 1) * 512],
                            start=(st == 0),
                            stop=(st == ST - 1),
                            skip_group_check=True,
                        )
                ctx_o = rows.tile([1, H], F32, tag="rows")
                nc.vector.tensor_scalar_mul(ctx_o[:], ctx_ps[:], rz[0:1, 0:1])
                nc.scalar.dma_start(out=ctx_out[b:b + 1, :], in_=ctx_o[:])

    nc.finalize()
    return nc


def _get_nc():
    if "nc" not in _BUILT:
        _BUILT["nc"] = _build()
    return _BUILT["nc"]


def kernel(decoder_hidden, encoder_outputs, Wq, Wk, We):
    decoder_hidden = np.ascontiguousarray(np.asarray(decoder_hidden, dtype=np.float32))
    encoder_outputs = np.ascontiguousarray(np.asarray(encoder_outputs, dtype=np.float32))
    Wq = np.ascontiguousarray(np.asarray(Wq, dtype=np.float32))
    Wk = np.ascontiguousarray(np.asarray(Wk, dtype=np.float32))
    We = np.ascontiguousarray(np.asarray(We, dtype=np.float32))

    nc = _get_nc()
    in_maps = []
    for c in range(NCORES):
        lo, hi = c * BPC, (c + 1) * BPC
        in_maps.append({
            "decoder_hidden": decoder_hidden[lo:hi],
            "encoder_outputs": encoder_outputs[lo:hi],
            "Wq": Wq,
            "Wk": Wk,
            "We": We,
        })
    res = run_bass_kernel_spmd(nc, in_maps, core_ids=list(range(NCORES)), trace=TRACE)
    if TRACE:
        _BUILT["last_result"] = res
    context = np.concatenate([res.results[c]["context"] for c in range(NCORES)], axis=0)
    attn = np.concatenate([res.results[c]["attn"] for c in range(NCORES)], axis=0)
    return (context, attn)
